# revision 1
# baseline (speedup 1.0000x reference)
"""Quaternion multi-head attention (nn_Attention_53395033424361) on 8 TRN2 NeuronCores.

Sharding: core = b*2 + hg  (b in 0..3 batches, hg in 0..1 head-groups of 4 heads).
Each core computes, for its batch b and its 4 heads, the attention output and a
partial output-projection y_part[b] (contraction over its heads' 384 features).
Host unshard: y[b] = y_part[core 2b] + y_part[core 2b+1] + bias.

All quaternion (Hamilton) structure is folded into host-assembled effective
weight matrices:
  - qkv qlinear              -> x @ W_eff,  W_eff [768, 2304] block-sign matrix
  - Hamilton score product   -> s_c = q @ (x @ K_c).T   (K_c: chunk-permuted/signed Wk)
  - Hamilton value product   -> o = sum_c softmax(s_c) @ (x @ V_c)
  - output qlinear           -> y = o_all @ Wp_eff + bp

On-device layout is fully "transposed" (features on partitions, tokens on the
free axis): scores are built as s_c^T [keys, tokens] so softmax sums over keys
arrive via an extra all-ones column in the AV matmul, and the per-token 1/r
normalizer is broadcast across partitions with a rank-1 matmul.
"""

import contextlib
import ctypes
import os
import sys
import types

import ml_dtypes
import numpy as np

import concourse.bass as bass
import concourse.mybir as mybir
import concourse.tile as tile
from concourse import bacc, bass_utils

B, N, DIM, H = 4, 1024, 768, 8
HD = DIM // H          # 96 head dim
QC = HD // 4           # 24 quaternion sub-chunk
NCORES = 8
HPC = H // 2           # heads per core (4)
DT = 6                 # 768 / 128 contraction tiles
F32 = mybir.dt.float32
BF16 = mybir.dt.bfloat16
MM_DT = mybir.dt.float32r  # fast fp32 matmul mode (1 cyc/row when N>=256)

_PROGRAM_CACHE = {}


# ----------------------------------------------------------------------------
# Host-side weight assembly
# ----------------------------------------------------------------------------

def _build_w_eff(wr, wi, wj, wk):
    row_r = np.concatenate([wr, wi, wj, wk], axis=1)
    row_i = np.concatenate([-wi, wr, -wk, wj], axis=1)
    row_j = np.concatenate([-wj, wk, wr, -wi], axis=1)
    row_k = np.concatenate([-wk, -wj, wi, wr], axis=1)
    return np.concatenate([row_r, row_i, row_j, row_k], axis=0)


def _k_variants(Wk):
    c = [Wk[:, i*QC:(i+1)*QC] for i in range(4)]
    return [
        np.concatenate([c[0], -c[1], -c[2], -c[3]], 1),
        np.concatenate([c[1], c[0], c[3], -c[2]], 1),
        np.concatenate([c[2], -c[3], c[0], c[1]], 1),
        np.concatenate([c[3], c[2], -c[1], c[0]], 1),
    ]


def _v_variants(Wv):
    c = [Wv[:, i*QC:(i+1)*QC] for i in range(4)]
    return [
        np.concatenate([c[0], c[1], c[2], c[3]], 1),
        np.concatenate([-c[1], c[0], -c[3], c[2]], 1),
        np.concatenate([-c[2], c[3], c[0], -c[1]], 1),
        np.concatenate([-c[3], -c[2], c[1], c[0]], 1),
    ]


def _host_prepare(inputs):
    """Returns (in_maps, bp) -- one input dict per core."""
    f32 = np.float32
    x = np.ascontiguousarray(np.asarray(inputs["x"], f32))
    W = _build_w_eff(*[np.asarray(inputs[f"wqkv_{c}"], f32) for c in "rijk"])
    Wp = _build_w_eff(*[np.asarray(inputs[f"wp_{c}"], f32) for c in "rijk"])
    bp = np.asarray(inputs["bp"], f32)

    def pad32(w):
        # [768, 96] -> [768, 128]: each 24-col chunk lands at a 32-col slot
        # (zero-filled) so on-device partition slices stay 32-aligned
        out = np.zeros((w.shape[0], 128), f32)
        for e in range(4):
            out[:, 32*e:32*e+QC] = w[:, QC*e:QC*(e+1)]
        return out

    # Per-head device weights:
    #  wa [768, 256]: [K_r(pad32) | q*scale(pad32)]; K_i/j/k built on device
    #  wv [768, 384]: [V_r | V_i | V_j | V_k]
    wa_heads, wv_heads = [], []
    for h in range(H):
        Wq = W[:, h*HD:(h+1)*HD] * f32(HD ** -0.5)
        Wk = W[:, DIM + h*HD: DIM + (h+1)*HD]
        Wv = W[:, 2*DIM + h*HD: 2*DIM + (h+1)*HD]
        wa_heads.append(np.concatenate(
            [pad32(_k_variants(Wk)[0]), pad32(Wq)], axis=1))
        wv_heads.append(np.concatenate(_v_variants(Wv), axis=1))

    def ptile(w):
        # [768, F] -> partition-major [128, 6*F] so the device DMA is contiguous
        f = w.shape[1]
        return np.ascontiguousarray(
            w.reshape(DT, 128, f).transpose(1, 0, 2).reshape(128, DT * f))

    in_maps = []
    for core in range(NCORES):
        b, hg = core // 2, core % 2
        hs = hg * HPC
        wp_c = Wp[hs*HD:(hs+HPC)*HD, :]                                # [384, 768]
        in_maps.append({
            "xt": ptile(x[b].T),                                       # [128, 6144]
            "wa": np.ascontiguousarray(np.concatenate(
                [ptile(wa_heads[hs+i]) for i in range(HPC)], axis=1)),  # [128, 4*2880]
            "wv": np.ascontiguousarray(np.concatenate(
                [ptile(wv_heads[hs+i]) for i in range(HPC)], axis=1)),  # [128, 4*2304]
            "wp": np.ascontiguousarray(
                wp_c.reshape(HPC, HD, DIM).transpose(1, 0, 2)
                .reshape(HD, HPC * DIM)),                              # [96, 3072]
        })
    return in_maps, bp


# ----------------------------------------------------------------------------
# Device program (SPMD -- identical on all 8 cores)
# ----------------------------------------------------------------------------

def _build_program():
    nc = bacc.Bacc("TRN2", target_bir_lowering=False, debug=False,
                   num_devices=NCORES)
    xt_d = nc.dram_tensor("xt", [128, DT * N], MM_DT, kind="ExternalInput").ap()
    wa_d = nc.dram_tensor("wa", [128, HPC * DT * 256], MM_DT, kind="ExternalInput").ap()
    wv_d = nc.dram_tensor("wv", [128, HPC * DT * 4 * HD], MM_DT, kind="ExternalInput").ap()
    wp_d = nc.dram_tensor("wp", [HD, HPC * DIM], MM_DT, kind="ExternalInput").ap()
    y_d = nc.dram_tensor("y", [N, DIM], F32, kind="ExternalOutput").ap()

    EXP = mybir.ActivationFunctionType.Exp

    with tile.TileContext(nc) as tc:
        with (
            tc.tile_pool(name="const", bufs=1) as cpool,
            tc.tile_pool(name="wstream", bufs=1) as wpool,
            tc.tile_pool(name="kvar", bufs=2) as kvar_pool,
            tc.tile_pool(name="vsb", bufs=2) as v_pool,
            tc.tile_pool(name="u", bufs=2) as u_pool,
            tc.tile_pool(name="small", bufs=2) as spool,
            tc.tile_pool(name="ysb", bufs=2) as y_pool,
            tc.tile_pool(name="ps_big", bufs=2, space="PSUM") as ps_big,
            tc.tile_pool(name="ps_o", bufs=2, space="PSUM") as ps_o,
            tc.tile_pool(name="ps_proj", bufs=2, space="PSUM") as ps_proj,
        ):
            # --- persistent tiles -------------------------------------------------
            xt_sb = cpool.tile([128, DT, N], MM_DT)
            nc.sync.dma_start(xt_sb[:], xt_d.rearrange("p (o t) -> p o t", o=DT))

            wp_sb = cpool.tile([128, HPC, DIM], MM_DT)
            nc.gpsimd.memset(wp_sb[HD:128, :, :].bitcast(F32), 0.0)
            nc.sync.dma_start(wp_sb[0:HD, :, :],
                              wp_d.rearrange("p (h g) -> p h g", h=HPC))

            # sel0/sel1: rank-1 selectors for the 1/r partition-broadcast matmul
            # (selector rows at partitions 0 and 32: engine APs must start at a
            # 32-aligned partition)
            sel = cpool.tile([128, 2, HD], MM_DT)
            nc.gpsimd.memset(sel[:].bitcast(F32), 0.0)
            nc.gpsimd.memset(sel[0:1, 0, :].bitcast(F32), 1.0)
            nc.gpsimd.memset(sel[32:33, 1, :].bitcast(F32), 1.0)

            # o^T accumulator for all 4 heads [96 feat, head, tokens]
            o_sb = cpool.tile([128, HPC, N], MM_DT)
            nc.gpsimd.memset(o_sb[HD:128, :, :].bitcast(F32), 0.0)

            for h in range(HPC):
                # --- stream this head's weights ----------------------------------
                wa_sb = wpool.tile([128, DT, 256], MM_DT, tag="wa")
                nc.sync.dma_start(
                    wa_sb[:],
                    wa_d[:, h*DT*256:(h+1)*DT*256]
                    .rearrange("p (o f) -> p o f", o=DT))
                wv_sb = wpool.tile([128, DT, 4 * HD], MM_DT, tag="wv")
                nc.sync.dma_start(
                    wv_sb[:],
                    wv_d[:, h*DT*4*HD:(h+1)*DT*4*HD]
                    .rearrange("p (o f) -> p o f", o=DT))

                # --- proj-A: transposed q / K_r features (32-padded chunks) ------
                # kvar_sb [128, 5, 1024]: block 0 = K_r^T, 1-3 = K_i/j/k^T
                # (built below from K_r), 4 = q^T
                kvar_sb = kvar_pool.tile([128, 5, N], MM_DT, tag="kvar")
                for blk in range(2):
                    dst_blk = 0 if blk == 0 else 4
                    for th in range(2):
                        psA = ps_proj.tile([128, 512], F32, tag="psp",
                                           name=f"psA_{h}_{blk}_{th}")
                        for d in range(DT):
                            nc.tensor.matmul(
                                psA[:, :],
                                lhsT=wa_sb[:, d, blk*128:(blk+1)*128],
                                rhs=xt_sb[:, d, th*512:(th+1)*512],
                                start=(d == 0), stop=(d == DT - 1))
                        nc.vector.tensor_copy(
                            kvar_sb[:, dst_blk, th*512:(th+1)*512], psA[:, :])
                # K_i/j/k from K_r: signed 32-row chunk moves (DVE).
                # (src_slot, sign) per dst slot:
                VAR_TABLE = [
                    [(1, -1.0), (0, 1.0), (3, -1.0), (2, 1.0)],   # K_i
                    [(2, -1.0), (3, 1.0), (0, 1.0), (1, -1.0)],   # K_j
                    [(3, -1.0), (2, -1.0), (1, 1.0), (0, 1.0)],   # K_k
                ]
                for v, table in enumerate(VAR_TABLE):
                    for t, (s, sign) in enumerate(table):
                        nc.vector.tensor_scalar_mul(
                            kvar_sb[32*t:32*t+32, 1 + v, :],
                            kvar_sb[32*s:32*s+32, 0, :].bitcast(F32),
                            sign)

                # --- proj-B: V-variants in token-partition layout ----------------
                # v_sb [128 keys, key-tile, comp, 98]: col 96 = ones (softmax sum)
                v_sb = v_pool.tile([128, 8, 4, 98], MM_DT, tag="vsb")
                nc.gpsimd.memset(v_sb[:, :, :, HD:HD+1].bitcast(F32), 1.0)
                for tt in range(8):
                    psB = ps_proj.tile([128, 512], F32, tag="psp",
                                       name=f"psB_{h}_{tt}")
                    for d in range(DT):
                        nc.tensor.matmul(
                            psB[:, 0:4*HD],
                            lhsT=xt_sb[:, d, tt*128:(tt+1)*128],
                            rhs=wv_sb[:, d, :],
                            start=(d == 0), stop=(d == DT - 1))
                    nc.vector.tensor_copy(
                        v_sb[:, tt, :, 0:HD],
                        psB[:, 0:4*HD].rearrange("p (c j) -> p c j", c=4))

                # --- attention ---------------------------------------------------
                # norm emission for block i is delayed until after block i+1's
                # first scores+exp, so the PE queue is not head-of-line blocked
                # on the (DVE) reciprocal chain.
                pending_norm = [None]
                oaccs = {}
                for th in range(2):
                    tok = slice(th*512, (th+1)*512)
                    oacc = spool.tile([128, 512], F32, tag="oacc",
                                      name=f"oacc_{h}_{th}")
                    oaccs[th] = oacc
                    for cp in range(2):
                        po = [ps_o.tile([128, 512], F32, tag="pso",
                                        name=f"po_{h}_{th}_{cp}_{ci}")
                              for ci in range(2)]
                        for kt in range(8):
                            psS = ps_big.tile([128, 1024], F32, tag="psb",
                                              name=f"psS_{h}_{th}_{cp}_{kt}")
                            for ci in range(2):
                                nc.tensor.matmul(
                                    psS[:, ci*512:(ci+1)*512],
                                    lhsT=kvar_sb[:, 2*cp+ci,
                                                 kt*128:(kt+1)*128],
                                    rhs=kvar_sb[:, 4, tok],
                                    start=True, stop=True)
                            u = u_pool.tile([128, 1024], MM_DT, tag="u",
                                            name=f"u_{h}_{th}_{cp}_{kt}")
                            nc.scalar.activation(u[:], psS[:], EXP)
                            if kt == 0 and pending_norm[0] is not None:
                                pending_norm[0]()
                                pending_norm[0] = None
                            for ci in range(2):
                                nc.tensor.matmul(
                                    po[ci][0:HD+1, :],
                                    lhsT=v_sb[:, kt, 2*cp+ci, 0:HD+1],
                                    rhs=u[:, ci*512:(ci+1)*512],
                                    start=(kt == 0), stop=(kt == 7))

                        def norm(th=th, cp=cp, po=po, tok=tok, h=h):
                            # softmax normalization: o += po[c][:96] * (1/r_c)
                            # bcast (r rows at partitions 0/32; cross-base COPY
                            # is HW-safe, cross-base reciprocal is not)
                            oacc = oaccs[th]
                            rp = spool.tile([128, 512], MM_DT, tag="rinv",
                                            name=f"rp_{h}_{th}_{cp}")
                            nc.gpsimd.memset(rp[:].bitcast(F32), 0.0)
                            for ci in range(2):
                                nc.vector.tensor_copy(
                                    rp[32*ci:32*ci+1, :], po[ci][HD:HD+1, :])
                            psR = ps_big.tile([128, 1024], F32, tag="psb",
                                              name=f"psR_{h}_{th}_{cp}")
                            for ci in range(2):
                                nc.tensor.matmul(
                                    psR[0:HD, ci*512:(ci+1)*512],
                                    lhsT=sel[:, ci, :],
                                    rhs=rp[:],
                                    start=True, stop=True)
                            rbc = spool.tile([128, 1024], F32, tag="rbc",
                                             name=f"rbc_{h}_{th}_{cp}")
                            nc.vector.reciprocal_approx_fast(
                                rbc[0:HD, :], psR[0:HD, :])
                            for ci in range(2):
                                idx = 2*cp + ci
                                if idx == 0:
                                    nc.vector.tensor_mul(
                                        oacc[0:HD, :],
                                        po[ci][0:HD, :],
                                        rbc[0:HD, ci*512:(ci+1)*512])
                                else:
                                    tmp = spool.tile(
                                        [128, 512], F32, tag="otmp",
                                        name=f"otmp_{h}_{th}_{cp}_{ci}")
                                    nc.vector.tensor_mul(
                                        tmp[0:HD, :], po[ci][0:HD, :],
                                        rbc[0:HD, ci*512:(ci+1)*512])
                                    dst = (o_sb[0:HD, h, tok] if idx == 3
                                           else oacc[0:HD, :])
                                    nc.vector.tensor_add(
                                        dst, oacc[0:HD, :], tmp[0:HD, :])

                        pending_norm[0] = norm
                if pending_norm[0] is not None:
                    pending_norm[0]()
                    pending_norm[0] = None

            # --- proj-C: partial output projection (contraction over heads) -----
            for tt in range(8):
                y_sb = y_pool.tile([128, DIM], F32, tag="ysb")
                for gh in range(2):
                    psY = ps_proj.tile([128, 512], F32, tag="psp",
                                       name=f"psY_{tt}_{gh}")
                    for hh in range(HPC):
                        nc.tensor.matmul(
                            psY[:, 0:384],
                            lhsT=o_sb[:, hh, tt*128:(tt+1)*128],
                            rhs=wp_sb[:, hh, gh*384:(gh+1)*384],
                            start=(hh == 0), stop=(hh == HPC - 1))
                    nc.vector.tensor_copy(y_sb[:, gh*384:(gh+1)*384],
                                          psY[:, 0:384])
                nc.sync.dma_start(y_d[tt*128:(tt+1)*128, :], y_sb[:])

    nc.compile()
    return nc


def _get_program():
    if "nc" not in _PROGRAM_CACHE:
        _PROGRAM_CACHE["nc"] = _build_program()
    return _PROGRAM_CACHE["nc"]


# ----------------------------------------------------------------------------
# NTFF profiling hook (axon containers without antenv.axon_hooks)
# ----------------------------------------------------------------------------

def _install_ntff_hook():
    """Provide antenv.axon_hooks backed by libaxon_pjrt.so so that
    run_bass_kernel_spmd(trace=True) can capture NTFF profiles under axon.
    Returns True if tracing is possible."""
    try:
        from antenv.axon_hooks import get_axon_ntff_profile_hook  # noqa: F401
        return True
    except ImportError:
        pass
    so_path = "/opt/axon/libaxon_pjrt.so"
    if not os.path.exists(so_path):
        return False
    lib = ctypes.CDLL(so_path)
    if not hasattr(lib, "axon_start_nrt_profile"):
        return False
    lib.axon_start_nrt_profile.argtypes = [
        ctypes.POINTER(ctypes.c_int64), ctypes.c_size_t]
    lib.axon_start_nrt_profile.restype = ctypes.c_int64
    lib.axon_stop_nrt_profile.argtypes = [ctypes.c_char_p]
    lib.axon_stop_nrt_profile.restype = ctypes.c_int64

    @contextlib.contextmanager
    def _hook(output_dir, device_ids):
        import jax
        jax.devices()
        if device_ids:
            ids = (ctypes.c_int64 * len(device_ids))(*device_ids)
            rc = lib.axon_start_nrt_profile(ids, len(device_ids))
        else:
            rc = lib.axon_start_nrt_profile(None, 0)
        if rc != 0:
            raise RuntimeError(f"axon_start_nrt_profile rc={rc}")
        try:
            yield
        finally:
            n = lib.axon_stop_nrt_profile(str(output_dir).encode())
            print(f"profile: {n} file(s) written to {output_dir}",
                  file=sys.stderr)

    mod = types.ModuleType("antenv.axon_hooks")
    _state = {"hook": _hook}
    mod.set_axon_ntff_profile_hook = lambda h: _state.__setitem__("hook", h)
    mod.get_axon_ntff_profile_hook = lambda: _state["hook"]
    sys.modules["antenv.axon_hooks"] = mod
    import antenv
    antenv.axon_hooks = mod
    return True


# ----------------------------------------------------------------------------
# Entry point
# ----------------------------------------------------------------------------

def kernel(trace=False, **inputs):
    nc = _get_program()
    in_maps, bp = _host_prepare(inputs)
    if trace:
        trace = _install_ntff_hook()
    res = bass_utils.run_bass_kernel_spmd(
        nc, in_maps, core_ids=list(range(NCORES)), trace=trace)
    y = np.empty((B, N, DIM), np.float32)
    for b in range(B):
        y[b] = res.results[2*b]["y"] + res.results[2*b+1]["y"] + bp
    if trace:
        kernel.last_results = res
    return y



# revision 8
# speedup vs baseline: 1.1024x; 1.1024x over previous
"""Quaternion multi-head attention (nn_Attention_53395033424361) on 8 TRN2 NeuronCores.

Sharding: core = b*2 + hg  (b in 0..3 batches, hg in 0..1 head-groups of 4 heads).
Each core computes, for its batch b and its 4 heads, the attention output and a
partial output-projection y_part[b] (contraction over its heads' 384 features).
Host unshard: y[b] = y_part[core 2b] + y_part[core 2b+1] + bias.

All quaternion (Hamilton) structure is folded into host-assembled effective
weight matrices:
  - qkv qlinear              -> x @ W_eff,  W_eff [768, 2304] block-sign matrix
  - Hamilton score product   -> s_c = q @ (x @ K_c).T   (K_c: chunk-permuted/signed Wk)
  - Hamilton value product   -> o = sum_c softmax(s_c) @ (x @ V_c)
  - output qlinear           -> y = o_all @ Wp_eff + bp

On-device layout is fully "transposed" (features on partitions, tokens on the
free axis): scores are built as s_c^T [keys, tokens] so softmax sums over keys
arrive via an extra all-ones column in the AV matmul, and the per-token 1/r
normalizer is broadcast across partitions with a rank-1 matmul.
"""

import contextlib
import ctypes
import os
import sys
import types

import ml_dtypes
import numpy as np

import concourse.bass as bass
import concourse.mybir as mybir
import concourse.tile as tile
from concourse import bacc, bass_utils

B, N, DIM, H = 4, 1024, 768, 8
HD = DIM // H          # 96 head dim
QC = HD // 4           # 24 quaternion sub-chunk
NCORES = 8
HPC = H // 2           # heads per core (4)
DT = 6                 # 768 / 128 contraction tiles
F32 = mybir.dt.float32
BF16 = mybir.dt.bfloat16
MM_DT = BF16  # bf16 matmuls: enables fast-weight-load + LDWEIGHTS pipelining

_PROGRAM_CACHE = {}


# ----------------------------------------------------------------------------
# Host-side weight assembly
# ----------------------------------------------------------------------------

def _build_w_eff(wr, wi, wj, wk):
    row_r = np.concatenate([wr, wi, wj, wk], axis=1)
    row_i = np.concatenate([-wi, wr, -wk, wj], axis=1)
    row_j = np.concatenate([-wj, wk, wr, -wi], axis=1)
    row_k = np.concatenate([-wk, -wj, wi, wr], axis=1)
    return np.concatenate([row_r, row_i, row_j, row_k], axis=0)


def _k_variants(Wk):
    c = [Wk[:, i*QC:(i+1)*QC] for i in range(4)]
    return [
        np.concatenate([c[0], -c[1], -c[2], -c[3]], 1),
        np.concatenate([c[1], c[0], c[3], -c[2]], 1),
        np.concatenate([c[2], -c[3], c[0], c[1]], 1),
        np.concatenate([c[3], c[2], -c[1], c[0]], 1),
    ]


def _v_variants(Wv):
    c = [Wv[:, i*QC:(i+1)*QC] for i in range(4)]
    return [
        np.concatenate([c[0], c[1], c[2], c[3]], 1),
        np.concatenate([-c[1], c[0], -c[3], c[2]], 1),
        np.concatenate([-c[2], c[3], c[0], -c[1]], 1),
        np.concatenate([-c[3], -c[2], c[1], c[0]], 1),
    ]


def _host_prepare(inputs):
    """Returns (in_maps, bp) -- one input dict per core."""
    f32 = np.float32
    x = np.ascontiguousarray(np.asarray(inputs["x"], f32))
    W = _build_w_eff(*[np.asarray(inputs[f"wqkv_{c}"], f32) for c in "rijk"])
    Wp = _build_w_eff(*[np.asarray(inputs[f"wp_{c}"], f32) for c in "rijk"])
    bp = np.asarray(inputs["bp"], f32)

    def pad32(w):
        # [768, 96] -> [768, 128]: each 24-col chunk lands at a 32-col slot
        # (zero-filled) so on-device partition slices stay 32-aligned
        out = np.zeros((w.shape[0], 128), f32)
        for e in range(4):
            out[:, 32*e:32*e+QC] = w[:, QC*e:QC*(e+1)]
        return out

    # Per-head device weights:
    #  wa [768, 256]: [K_r(pad32) | q*scale(pad32)]; K_i/j/k built on device
    #  wv [768, 384]: [V_r | V_i | V_j | V_k]
    wa_heads, wv_heads = [], []
    for h in range(H):
        Wq = W[:, h*HD:(h+1)*HD] * f32(HD ** -0.5)
        Wk = W[:, DIM + h*HD: DIM + (h+1)*HD]
        Wv = W[:, 2*DIM + h*HD: 2*DIM + (h+1)*HD]
        wa_heads.append(np.concatenate(
            [pad32(_k_variants(Wk)[0]), pad32(Wq)], axis=1))
        wv_heads.append(np.concatenate(_v_variants(Wv), axis=1))

    def ptile(w):
        # [768, F] -> partition-major [128, 6*F] so the device DMA is contiguous
        f = w.shape[1]
        return np.ascontiguousarray(
            w.reshape(DT, 128, f).transpose(1, 0, 2).reshape(128, DT * f))

    bf16 = ml_dtypes.bfloat16
    in_maps = []
    for core in range(NCORES):
        b, hg = core // 2, core % 2
        hs = hg * HPC
        wp_c = Wp[hs*HD:(hs+HPC)*HD, :]                                # [384, 768]
        in_maps.append({
            "xt": ptile(x[b].T).astype(bf16),                          # [128, 6144]
            "wa": np.ascontiguousarray(np.concatenate(
                [ptile(wa_heads[hs+i]) for i in range(HPC)],
                axis=1)).astype(bf16),                                 # [128, 4*2880]
            "wv": np.ascontiguousarray(np.concatenate(
                [ptile(wv_heads[hs+i]) for i in range(HPC)],
                axis=1)).astype(bf16),                                 # [128, 4*2304]
            "wp": np.ascontiguousarray(
                wp_c.reshape(HPC, HD, DIM).transpose(1, 0, 2)
                .reshape(HD, HPC * DIM)).astype(bf16),                 # [96, 3072]
        })
    return in_maps, bp


# ----------------------------------------------------------------------------
# Device program (SPMD -- identical on all 8 cores)
# ----------------------------------------------------------------------------

def _build_program():
    nc = bacc.Bacc("TRN2", target_bir_lowering=False, debug=False,
                   num_devices=NCORES)
    xt_d = nc.dram_tensor("xt", [128, DT * N], MM_DT, kind="ExternalInput").ap()
    wa_d = nc.dram_tensor("wa", [128, HPC * DT * 256], MM_DT, kind="ExternalInput").ap()
    wv_d = nc.dram_tensor("wv", [128, HPC * DT * 4 * HD], MM_DT, kind="ExternalInput").ap()
    wp_d = nc.dram_tensor("wp", [HD, HPC * DIM], MM_DT, kind="ExternalInput").ap()
    y_d = nc.dram_tensor("y", [N, DIM], F32, kind="ExternalOutput").ap()

    EXP = mybir.ActivationFunctionType.Exp

    with tile.TileContext(nc) as tc:
        with (
            tc.tile_pool(name="const", bufs=1) as cpool,
            tc.tile_pool(name="wstream", bufs=1) as wpool,
            tc.tile_pool(name="kvar", bufs=2) as kvar_pool,
            tc.tile_pool(name="vsb", bufs=2) as v_pool,
            tc.tile_pool(name="u", bufs=2) as u_pool,
            tc.tile_pool(name="small", bufs=2) as spool,
            tc.tile_pool(name="ysb", bufs=2) as y_pool,
            tc.tile_pool(name="ps_big", bufs=2, space="PSUM") as ps_big,
            tc.tile_pool(name="ps_o", bufs=2, space="PSUM") as ps_o,
            tc.tile_pool(name="ps_proj", bufs=2, space="PSUM") as ps_proj,
        ):
            # --- persistent tiles -------------------------------------------------
            xt_sb = cpool.tile([128, DT, N], MM_DT)
            nc.sync.dma_start(xt_sb[:], xt_d.rearrange("p (o t) -> p o t", o=DT))

            wp_sb = cpool.tile([128, HPC, DIM], MM_DT)
            nc.gpsimd.memset(wp_sb[HD:128, :, :], 0.0)
            nc.sync.dma_start(wp_sb[0:HD, :, :],
                              wp_d.rearrange("p (h g) -> p h g", h=HPC))

            # sel0/sel1: rank-1 selectors for the 1/r partition-broadcast matmul
            # (selector rows at partitions 0 and 32: engine APs must start at a
            # 32-aligned partition)
            sel = cpool.tile([128, 2, HD], MM_DT)
            nc.gpsimd.memset(sel[:], 0.0)
            nc.gpsimd.memset(sel[0:1, 0, :], 1.0)
            nc.gpsimd.memset(sel[32:33, 1, :], 1.0)

            # o^T accumulator for all 4 heads [96 feat, head, tokens]
            o_sb = cpool.tile([128, HPC, N], MM_DT)
            nc.gpsimd.memset(o_sb[HD:128, :, :], 0.0)

            for h in range(HPC):
                # --- stream this head's weights ----------------------------------
                wa_sb = wpool.tile([128, DT, 256], MM_DT, tag="wa")
                nc.sync.dma_start(
                    wa_sb[:],
                    wa_d[:, h*DT*256:(h+1)*DT*256]
                    .rearrange("p (o f) -> p o f", o=DT))
                wv_sb = wpool.tile([128, DT, 4 * HD], MM_DT, tag="wv")
                nc.sync.dma_start(
                    wv_sb[:],
                    wv_d[:, h*DT*4*HD:(h+1)*DT*4*HD]
                    .rearrange("p (o f) -> p o f", o=DT))

                # --- proj-A: transposed q / K_r features (32-padded chunks) ------
                # kvar_sb [128, 5, 1024]: block 0 = K_r^T, 1-3 = K_i/j/k^T
                # (built below from K_r), 4 = q^T
                kvar_sb = kvar_pool.tile([128, 5, N], MM_DT, tag="kvar")
                for blk in range(2):
                    dst_blk = 0 if blk == 0 else 4
                    for th in range(2):
                        psA = ps_proj.tile([128, 512], F32, tag="psp",
                                           name=f"psA_{h}_{blk}_{th}")
                        for d in range(DT):
                            nc.tensor.matmul(
                                psA[:, :],
                                lhsT=wa_sb[:, d, blk*128:(blk+1)*128],
                                rhs=xt_sb[:, d, th*512:(th+1)*512],
                                start=(d == 0), stop=(d == DT - 1))
                        nc.vector.tensor_copy(
                            kvar_sb[:, dst_blk, th*512:(th+1)*512], psA[:, :])
                # K_i/j/k from K_r: signed 32-row chunk moves (DVE).
                # (src_slot, sign) per dst slot:
                VAR_TABLE = [
                    [(1, -1.0), (0, 1.0), (3, -1.0), (2, 1.0)],   # K_i
                    [(2, -1.0), (3, 1.0), (0, 1.0), (1, -1.0)],   # K_j
                    [(3, -1.0), (2, -1.0), (1, 1.0), (0, 1.0)],   # K_k
                ]
                for v, table in enumerate(VAR_TABLE):
                    for t, (s, sign) in enumerate(table):
                        nc.vector.tensor_scalar_mul(
                            kvar_sb[32*t:32*t+32, 1 + v, :],
                            kvar_sb[32*s:32*s+32, 0, :],
                            sign)

                # --- proj-B: V-variants in token-partition layout ----------------
                # v_sb [128 keys, key-tile, comp, 98]: col 96 = ones (softmax sum)
                v_sb = v_pool.tile([128, 8, 4, 98], MM_DT, tag="vsb")
                nc.gpsimd.memset(v_sb[:, :, :, HD:HD+1], 1.0)
                for tt in range(8):
                    psB = ps_proj.tile([128, 512], F32, tag="psp",
                                       name=f"psB_{h}_{tt}")
                    for d in range(DT):
                        nc.tensor.matmul(
                            psB[:, 0:4*HD],
                            lhsT=xt_sb[:, d, tt*128:(tt+1)*128],
                            rhs=wv_sb[:, d, :],
                            start=(d == 0), stop=(d == DT - 1))
                    nc.vector.tensor_copy(
                        v_sb[:, tt, :, 0:HD],
                        psB[:, 0:4*HD].rearrange("p (c j) -> p c j", c=4))

                # --- attention ---------------------------------------------------
                # norm emission for block i is delayed until after block i+1's
                # first scores+exp, so the PE queue is not head-of-line blocked
                # on the (DVE) reciprocal chain.
                pending_norm = [None]
                oaccs = {}
                for th in range(2):
                    tok = slice(th*512, (th+1)*512)
                    oacc = spool.tile([128, 512], F32, tag="oacc",
                                      name=f"oacc_{h}_{th}")
                    oaccs[th] = oacc
                    for cp in range(2):
                        po = [ps_o.tile([128, 512], F32, tag="pso",
                                        name=f"po_{h}_{th}_{cp}_{ci}")
                              for ci in range(2)]
                        for kt in range(8):
                            psS = ps_big.tile([128, 1024], F32, tag="psb",
                                              name=f"psS_{h}_{th}_{cp}_{kt}")
                            for ci in range(2):
                                nc.tensor.matmul(
                                    psS[:, ci*512:(ci+1)*512],
                                    lhsT=kvar_sb[:, 2*cp+ci,
                                                 kt*128:(kt+1)*128],
                                    rhs=kvar_sb[:, 4, tok],
                                    start=True, stop=True)
                            u = u_pool.tile([128, 1024], MM_DT, tag="u",
                                            name=f"u_{h}_{th}_{cp}_{kt}")
                            nc.scalar.activation(u[:], psS[:], EXP)
                            if kt == 0 and pending_norm[0] is not None:
                                pending_norm[0]()
                                pending_norm[0] = None
                            for ci in range(2):
                                nc.tensor.matmul(
                                    po[ci][0:HD+1, :],
                                    lhsT=v_sb[:, kt, 2*cp+ci, 0:HD+1],
                                    rhs=u[:, ci*512:(ci+1)*512],
                                    start=(kt == 0), stop=(kt == 7))

                        def norm(th=th, cp=cp, po=po, tok=tok, h=h):
                            # softmax normalization: o += po[c][:96] * (1/r_c)
                            # bcast (r rows at partitions 0/32; cross-base COPY
                            # is HW-safe, cross-base reciprocal is not)
                            oacc = oaccs[th]
                            rp = spool.tile([128, 512], MM_DT, tag="rinv",
                                            name=f"rp_{h}_{th}_{cp}")
                            nc.gpsimd.memset(rp[:], 0.0)
                            for ci in range(2):
                                nc.vector.tensor_copy(
                                    rp[32*ci:32*ci+1, :], po[ci][HD:HD+1, :])
                            psR = ps_big.tile([128, 1024], F32, tag="psb",
                                              name=f"psR_{h}_{th}_{cp}")
                            for ci in range(2):
                                nc.tensor.matmul(
                                    psR[0:HD, ci*512:(ci+1)*512],
                                    lhsT=sel[:, ci, :],
                                    rhs=rp[:],
                                    start=True, stop=True)
                            rbc = spool.tile([128, 1024], F32, tag="rbc",
                                             name=f"rbc_{h}_{th}_{cp}")
                            nc.vector.reciprocal_approx_fast(
                                rbc[0:HD, :], psR[0:HD, :])
                            for ci in range(2):
                                idx = 2*cp + ci
                                if idx == 0:
                                    nc.vector.tensor_mul(
                                        oacc[0:HD, :],
                                        po[ci][0:HD, :],
                                        rbc[0:HD, ci*512:(ci+1)*512])
                                else:
                                    tmp = spool.tile(
                                        [128, 512], F32, tag="otmp",
                                        name=f"otmp_{h}_{th}_{cp}_{ci}")
                                    nc.vector.tensor_mul(
                                        tmp[0:HD, :], po[ci][0:HD, :],
                                        rbc[0:HD, ci*512:(ci+1)*512])
                                    dst = (o_sb[0:HD, h, tok] if idx == 3
                                           else oacc[0:HD, :])
                                    nc.vector.tensor_add(
                                        dst, oacc[0:HD, :], tmp[0:HD, :])

                        pending_norm[0] = norm
                if pending_norm[0] is not None:
                    pending_norm[0]()
                    pending_norm[0] = None

            # --- proj-C: partial output projection (contraction over heads) -----
            for tt in range(8):
                y_sb = y_pool.tile([128, DIM], F32, tag="ysb")
                for gh in range(2):
                    psY = ps_proj.tile([128, 512], F32, tag="psp",
                                       name=f"psY_{tt}_{gh}")
                    for hh in range(HPC):
                        nc.tensor.matmul(
                            psY[:, 0:384],
                            lhsT=o_sb[:, hh, tt*128:(tt+1)*128],
                            rhs=wp_sb[:, hh, gh*384:(gh+1)*384],
                            start=(hh == 0), stop=(hh == HPC - 1))
                    nc.vector.tensor_copy(y_sb[:, gh*384:(gh+1)*384],
                                          psY[:, 0:384])
                nc.sync.dma_start(y_d[tt*128:(tt+1)*128, :], y_sb[:])

    nc.compile()
    return nc


def _get_program():
    if "nc" not in _PROGRAM_CACHE:
        _PROGRAM_CACHE["nc"] = _build_program()
    return _PROGRAM_CACHE["nc"]


# ----------------------------------------------------------------------------
# NTFF profiling hook (axon containers without antenv.axon_hooks)
# ----------------------------------------------------------------------------

def _install_ntff_hook():
    """Provide antenv.axon_hooks backed by libaxon_pjrt.so so that
    run_bass_kernel_spmd(trace=True) can capture NTFF profiles under axon.
    Returns True if tracing is possible."""
    try:
        from antenv.axon_hooks import get_axon_ntff_profile_hook  # noqa: F401
        return True
    except ImportError:
        pass
    so_path = "/opt/axon/libaxon_pjrt.so"
    if not os.path.exists(so_path):
        return False
    lib = ctypes.CDLL(so_path)
    if not hasattr(lib, "axon_start_nrt_profile"):
        return False
    lib.axon_start_nrt_profile.argtypes = [
        ctypes.POINTER(ctypes.c_int64), ctypes.c_size_t]
    lib.axon_start_nrt_profile.restype = ctypes.c_int64
    lib.axon_stop_nrt_profile.argtypes = [ctypes.c_char_p]
    lib.axon_stop_nrt_profile.restype = ctypes.c_int64

    @contextlib.contextmanager
    def _hook(output_dir, device_ids):
        import jax
        jax.devices()
        if device_ids:
            ids = (ctypes.c_int64 * len(device_ids))(*device_ids)
            rc = lib.axon_start_nrt_profile(ids, len(device_ids))
        else:
            rc = lib.axon_start_nrt_profile(None, 0)
        if rc != 0:
            raise RuntimeError(f"axon_start_nrt_profile rc={rc}")
        try:
            yield
        finally:
            n = lib.axon_stop_nrt_profile(str(output_dir).encode())
            print(f"profile: {n} file(s) written to {output_dir}",
                  file=sys.stderr)

    mod = types.ModuleType("antenv.axon_hooks")
    _state = {"hook": _hook}
    mod.set_axon_ntff_profile_hook = lambda h: _state.__setitem__("hook", h)
    mod.get_axon_ntff_profile_hook = lambda: _state["hook"]
    sys.modules["antenv.axon_hooks"] = mod
    import antenv
    antenv.axon_hooks = mod
    return True


# ----------------------------------------------------------------------------
# Entry point
# ----------------------------------------------------------------------------

def kernel(trace=False, **inputs):
    nc = _get_program()
    in_maps, bp = _host_prepare(inputs)
    if trace:
        trace = _install_ntff_hook()
    res = bass_utils.run_bass_kernel_spmd(
        nc, in_maps, core_ids=list(range(NCORES)), trace=trace)
    y = np.empty((B, N, DIM), np.float32)
    for b in range(B):
        y[b] = res.results[2*b]["y"] + res.results[2*b+1]["y"] + bp
    if trace:
        kernel.last_results = res
    return y



# revision 21
# speedup vs baseline: 1.1217x; 1.0174x over previous
"""Quaternion multi-head attention (nn_Attention_53395033424361) on 8 TRN2 NeuronCores.

Sharding: core = b*2 + hg  (b in 0..3 batches, hg in 0..1 head-groups of 4 heads).
Each core computes, for its batch b and its 4 heads, the attention output and a
partial output-projection y_part[b] (contraction over its heads' 384 features).
Host unshard: y[b] = y_part[core 2b] + y_part[core 2b+1] + bias.

All quaternion (Hamilton) structure is folded into host-assembled effective
weight matrices:
  - qkv qlinear              -> x @ W_eff,  W_eff [768, 2304] block-sign matrix
  - Hamilton score product   -> s_c = q @ (x @ K_c).T   (K_c: chunk-permuted/signed Wk)
  - Hamilton value product   -> o = sum_c softmax(s_c) @ (x @ V_c)
  - output qlinear           -> y = o_all @ Wp_eff + bp

On-device layout is fully "transposed" (features on partitions, tokens on the
free axis): scores are built as s_c^T [keys, tokens] so softmax sums over keys
arrive via an extra all-ones column in the AV matmul, and the per-token 1/r
normalizer is broadcast across partitions with a rank-1 matmul.
"""

import contextlib
import ctypes
import os
import sys
import types

import ml_dtypes
import numpy as np

import concourse.bass as bass
import concourse.mybir as mybir
import concourse.tile as tile
from concourse import bacc, bass_utils

B, N, DIM, H = 4, 1024, 768, 8
HD = DIM // H          # 96 head dim
QC = HD // 4           # 24 quaternion sub-chunk
NCORES = 8
HPC = H // 2           # heads per core (4)
DT = 6                 # 768 / 128 contraction tiles
F32 = mybir.dt.float32
BF16 = mybir.dt.bfloat16
MM_DT = BF16  # bf16 matmuls: enables fast-weight-load + LDWEIGHTS pipelining

_PROGRAM_CACHE = {}


# ----------------------------------------------------------------------------
# Host-side weight assembly
# ----------------------------------------------------------------------------

def _build_w_eff(wr, wi, wj, wk):
    row_r = np.concatenate([wr, wi, wj, wk], axis=1)
    row_i = np.concatenate([-wi, wr, -wk, wj], axis=1)
    row_j = np.concatenate([-wj, wk, wr, -wi], axis=1)
    row_k = np.concatenate([-wk, -wj, wi, wr], axis=1)
    return np.concatenate([row_r, row_i, row_j, row_k], axis=0)


def _k_variants(Wk):
    c = [Wk[:, i*QC:(i+1)*QC] for i in range(4)]
    return [
        np.concatenate([c[0], -c[1], -c[2], -c[3]], 1),
        np.concatenate([c[1], c[0], c[3], -c[2]], 1),
        np.concatenate([c[2], -c[3], c[0], c[1]], 1),
        np.concatenate([c[3], c[2], -c[1], c[0]], 1),
    ]


def _v_variants(Wv):
    c = [Wv[:, i*QC:(i+1)*QC] for i in range(4)]
    return [
        np.concatenate([c[0], c[1], c[2], c[3]], 1),
        np.concatenate([-c[1], c[0], -c[3], c[2]], 1),
        np.concatenate([-c[2], c[3], c[0], -c[1]], 1),
        np.concatenate([-c[3], -c[2], c[1], c[0]], 1),
    ]


def _host_prepare(inputs):
    """Returns (in_maps, bp) -- one input dict per core."""
    f32 = np.float32
    x = np.ascontiguousarray(np.asarray(inputs["x"], f32))
    W = _build_w_eff(*[np.asarray(inputs[f"wqkv_{c}"], f32) for c in "rijk"])
    Wp = _build_w_eff(*[np.asarray(inputs[f"wp_{c}"], f32) for c in "rijk"])
    bp = np.asarray(inputs["bp"], f32)

    def pad32(w):
        # [768, 96] -> [768, 128]: each 24-col chunk lands at a 32-col slot
        # (zero-filled) so on-device partition slices stay 32-aligned
        out = np.zeros((w.shape[0], 128), f32)
        for e in range(4):
            out[:, 32*e:32*e+QC] = w[:, QC*e:QC*(e+1)]
        return out

    # Per-head device weights:
    #  wa [768, 256]: [K_r(pad32) | q*scale(pad32)]; K_i/j/k built on device
    #  wv [768, 384]: [V_r | V_i | V_j | V_k]
    wa_heads, wv_heads = [], []
    for h in range(H):
        Wq = W[:, h*HD:(h+1)*HD] * f32(HD ** -0.5)
        Wk = W[:, DIM + h*HD: DIM + (h+1)*HD]
        Wv = W[:, 2*DIM + h*HD: 2*DIM + (h+1)*HD]
        wa_heads.append(np.concatenate(
            [pad32(_k_variants(Wk)[0]), pad32(Wq)], axis=1))
        wv_heads.append(np.concatenate(_v_variants(Wv), axis=1))

    def ptile(w):
        # [768, F] -> partition-major [128, 6*F] so the device DMA is contiguous
        f = w.shape[1]
        return np.ascontiguousarray(
            w.reshape(DT, 128, f).transpose(1, 0, 2).reshape(128, DT * f))

    bf16 = ml_dtypes.bfloat16
    in_maps = []
    for core in range(NCORES):
        b, hg = core // 2, core % 2
        hs = hg * HPC
        wp_c = Wp[hs*HD:(hs+HPC)*HD, :]                                # [384, 768]
        in_maps.append({
            "xt": ptile(x[b].T).astype(bf16),                          # [128, 6144]
            "wa": np.ascontiguousarray(np.concatenate(
                [ptile(wa_heads[hs+i]) for i in range(HPC)],
                axis=1)).astype(bf16),                                 # [128, 4*2880]
            "wv": np.ascontiguousarray(np.concatenate(
                [ptile(wv_heads[hs+i]) for i in range(HPC)],
                axis=1)).astype(bf16),                                 # [128, 4*2304]
            "wp": np.ascontiguousarray(
                wp_c.reshape(HPC, HD, DIM).transpose(1, 0, 2)
                .reshape(HD, HPC * DIM)).astype(bf16),                 # [96, 3072]
        })
    return in_maps, bp


# ----------------------------------------------------------------------------
# Device program (SPMD -- identical on all 8 cores)
# ----------------------------------------------------------------------------

def _build_program():
    nc = bacc.Bacc("TRN2", target_bir_lowering=False, debug=False,
                   num_devices=NCORES)
    xt_d = nc.dram_tensor("xt", [128, DT * N], MM_DT, kind="ExternalInput").ap()
    wa_d = nc.dram_tensor("wa", [128, HPC * DT * 256], MM_DT, kind="ExternalInput").ap()
    wv_d = nc.dram_tensor("wv", [128, HPC * DT * 4 * HD], MM_DT, kind="ExternalInput").ap()
    wp_d = nc.dram_tensor("wp", [HD, HPC * DIM], MM_DT, kind="ExternalInput").ap()
    y_d = nc.dram_tensor("y", [N, DIM], BF16, kind="ExternalOutput").ap()

    EXP = mybir.ActivationFunctionType.Exp

    with tile.TileContext(nc) as tc:
        with (
            tc.tile_pool(name="const", bufs=1) as cpool,
            tc.tile_pool(name="wstream", bufs=2) as wpool,
            tc.tile_pool(name="kvar", bufs=2) as kvar_pool,
            tc.tile_pool(name="vsb", bufs=2) as v_pool,
            tc.tile_pool(name="u", bufs=2) as u_pool,
            tc.tile_pool(name="small", bufs=2) as spool,
            tc.tile_pool(name="ysb", bufs=2) as y_pool,
            tc.tile_pool(name="ps_big", bufs=2, space="PSUM") as ps_big,
            tc.tile_pool(name="ps_o", bufs=2, space="PSUM") as ps_o,
            tc.tile_pool(name="ps_proj", bufs=2, space="PSUM") as ps_proj,
        ):
            # --- persistent tiles -------------------------------------------------
            xt_sb = cpool.tile([128, DT, N], MM_DT)
            nc.sync.dma_start(xt_sb[:], xt_d.rearrange("p (o t) -> p o t", o=DT))

            wp_sb = cpool.tile([128, HPC, DIM], MM_DT)
            nc.gpsimd.memset(wp_sb[HD:128, :, :], 0.0)
            nc.sync.dma_start(wp_sb[0:HD, :, :],
                              wp_d.rearrange("p (h g) -> p h g", h=HPC))

            # sel0/sel1: rank-1 selectors for the 1/r partition-broadcast matmul
            # (selector rows at partitions 0 and 32: engine APs must start at a
            # 32-aligned partition)
            sel = cpool.tile([128, 2, HD], MM_DT)
            nc.gpsimd.memset(sel[:], 0.0)
            nc.gpsimd.memset(sel[0:1, 0, :], 1.0)
            nc.gpsimd.memset(sel[32:33, 1, :], 1.0)

            # o^T accumulator for all 4 heads [96 feat, head, tokens]
            o_sb = cpool.tile([128, HPC, N], MM_DT)
            nc.gpsimd.memset(o_sb[HD:128, :, :], 0.0)

            def emit_projC(tts):
                # partial output projection (contraction over heads) for the
                # given token tiles; requires o_sb[:, :, tt-slice] complete
                for tt in tts:
                    y_sb = y_pool.tile([128, DIM], BF16, tag="ysb")
                    for gh in range(2):
                        psY = ps_proj.tile([128, 512], F32, tag="psp",
                                           name=f"psY_{tt}_{gh}")
                        for hh in range(HPC):
                            nc.tensor.matmul(
                                psY[:, 0:384],
                                lhsT=o_sb[:, hh, tt*128:(tt+1)*128],
                                rhs=wp_sb[:, hh, gh*384:(gh+1)*384],
                                start=(hh == 0), stop=(hh == HPC - 1))
                        nc.vector.tensor_copy(y_sb[:, gh*384:(gh+1)*384],
                                              psY[:, 0:384])
                    nc.sync.dma_start(y_d[tt*128:(tt+1)*128, :], y_sb[:])

            for h in range(HPC):
                # --- stream this head's weights ----------------------------------
                wa_sb = wpool.tile([128, DT, 256], MM_DT, tag="wa")
                nc.sync.dma_start(
                    wa_sb[:],
                    wa_d[:, h*DT*256:(h+1)*DT*256]
                    .rearrange("p (o f) -> p o f", o=DT))
                wv_sb = wpool.tile([128, DT, 4 * HD], MM_DT, tag="wv")
                nc.sync.dma_start(
                    wv_sb[:],
                    wv_d[:, h*DT*4*HD:(h+1)*DT*4*HD]
                    .rearrange("p (o f) -> p o f", o=DT))

                # --- proj-A: transposed q / K_r features (32-padded chunks) ------
                # kvar_sb [128, 5, 1024]: block 0 = K_r^T, 1-3 = K_i/j/k^T
                # (built below from K_r), 4 = q^T
                kvar_sb = kvar_pool.tile([128, 5, N], MM_DT, tag="kvar")
                for blk in range(2):
                    dst_blk = 0 if blk == 0 else 4
                    for th in range(2):
                        psA = ps_proj.tile([128, 512], F32, tag="psp",
                                           name=f"psA_{h}_{blk}_{th}")
                        for d in range(DT):
                            nc.tensor.matmul(
                                psA[:, :],
                                lhsT=wa_sb[:, d, blk*128:(blk+1)*128],
                                rhs=xt_sb[:, d, th*512:(th+1)*512],
                                start=(d == 0), stop=(d == DT - 1))
                        nc.vector.tensor_copy(
                            kvar_sb[:, dst_blk, th*512:(th+1)*512], psA[:, :])
                # K_i/j/k from K_r: signed 32-row chunk moves (DVE).
                # (src_slot, sign) per dst slot:
                VAR_TABLE = [
                    [(1, -1.0), (0, 1.0), (3, -1.0), (2, 1.0)],   # K_i
                    [(2, -1.0), (3, 1.0), (0, 1.0), (1, -1.0)],   # K_j
                    [(3, -1.0), (2, -1.0), (1, 1.0), (0, 1.0)],   # K_k
                ]
                for v, table in enumerate(VAR_TABLE):
                    for t, (s, sign) in enumerate(table):
                        nc.vector.tensor_scalar_mul(
                            kvar_sb[32*t:32*t+32, 1 + v, :],
                            kvar_sb[32*s:32*s+32, 0, :],
                            sign)

                # --- proj-B: V-variants in token-partition layout ----------------
                # v_sb [128 keys, key-tile, comp, 98]: col 96 = ones (softmax sum)
                v_sb = v_pool.tile([128, 8, 4, 98], MM_DT, tag="vsb")
                nc.gpsimd.memset(v_sb[:, :, :, HD:HD+1], 1.0)
                for tt in range(8):
                    psB = ps_proj.tile([128, 512], F32, tag="psp",
                                       name=f"psB_{h}_{tt}")
                    for d in range(DT):
                        nc.tensor.matmul(
                            psB[:, 0:4*HD],
                            lhsT=xt_sb[:, d, tt*128:(tt+1)*128],
                            rhs=wv_sb[:, d, :],
                            start=(d == 0), stop=(d == DT - 1))
                    nc.vector.tensor_copy(
                        v_sb[:, tt, :, 0:HD],
                        psB[:, 0:4*HD].rearrange("p (c j) -> p c j", c=4))

                # --- attention ---------------------------------------------------
                # norm emission for block i is delayed until after block i+1's
                # first scores+exp, so the PE queue is not head-of-line blocked
                # on the (DVE) reciprocal chain.
                pending_norm = [None]
                oaccs = {}
                for th in range(2):
                    tok = slice(th*512, (th+1)*512)
                    oacc = spool.tile([128, 512], BF16, tag="oacc",
                                      name=f"oacc_{h}_{th}")
                    oaccs[th] = oacc
                    for cp in range(2):
                        po = [ps_o.tile([128, 512], F32, tag="pso",
                                        name=f"po_{h}_{th}_{cp}_{ci}")
                              for ci in range(2)]
                        for kt in range(8):
                            psS = ps_big.tile([128, 1024], F32, tag="psb",
                                              name=f"psS_{h}_{th}_{cp}_{kt}")
                            for ci in range(2):
                                nc.tensor.matmul(
                                    psS[:, ci*512:(ci+1)*512],
                                    lhsT=kvar_sb[:, 2*cp+ci,
                                                 kt*128:(kt+1)*128],
                                    rhs=kvar_sb[:, 4, tok],
                                    start=True, stop=True)
                            u = u_pool.tile([128, 1024], MM_DT, tag="u",
                                            name=f"u_{h}_{th}_{cp}_{kt}")
                            nc.scalar.activation(u[:], psS[:], EXP)
                            if kt == 0 and pending_norm[0] is not None:
                                pending_norm[0]()
                                pending_norm[0] = None
                                if h == HPC - 1 and th == 1 and cp == 0:
                                    # o_sb[:, :, 0:512] now complete for all
                                    # heads: overlap half of proj-C with the
                                    # last head's second token-half
                                    emit_projC(range(4))
                            for ci in range(2):
                                nc.tensor.matmul(
                                    po[ci][0:HD+1, :],
                                    lhsT=v_sb[:, kt, 2*cp+ci, 0:HD+1],
                                    rhs=u[:, ci*512:(ci+1)*512],
                                    start=(kt == 0), stop=(kt == 7))

                        # po -> SBUF right away (gpsimd) so the PSUM bank
                        # frees before the (deferred) norm arithmetic runs
                        pcp = spool.tile([128, 2, 512], F32, tag="pcp",
                                         name=f"pcp_{h}_{th}_{cp}")
                        for ci in range(2):
                            nc.vector.tensor_copy(pcp[0:HD+1, ci, :],
                                                  po[ci][0:HD+1, :])

                        def norm(th=th, cp=cp, pcp=pcp, tok=tok, h=h):
                            # softmax normalization: o += pcp[c][:96] * (1/r_c)
                            # bcast (r rows at partitions 0/32; cross-base COPY
                            # is HW-safe, cross-base reciprocal is not)
                            oacc = oaccs[th]
                            rp = spool.tile([128, 512], MM_DT, tag="rinv",
                                            name=f"rp_{h}_{th}_{cp}")
                            nc.gpsimd.memset(rp[:], 0.0)
                            for ci in range(2):
                                nc.vector.tensor_copy(
                                    rp[32*ci:32*ci+1, :], pcp[HD:HD+1, ci, :])
                            psR = ps_big.tile([128, 1024], F32, tag="psb",
                                              name=f"psR_{h}_{th}_{cp}")
                            for ci in range(2):
                                nc.tensor.matmul(
                                    psR[0:HD, ci*512:(ci+1)*512],
                                    lhsT=sel[:, ci, :],
                                    rhs=rp[:],
                                    start=True, stop=True)
                            rbc = spool.tile([128, 1024], F32, tag="rbc",
                                             name=f"rbc_{h}_{th}_{cp}")
                            nc.vector.reciprocal_approx_fast(
                                rbc[0:HD, :], psR[0:HD, :])
                            for ci in range(2):
                                idx = 2*cp + ci
                                if idx == 0:
                                    nc.vector.tensor_mul(
                                        oacc[0:HD, :],
                                        pcp[0:HD, ci, :],
                                        rbc[0:HD, ci*512:(ci+1)*512])
                                else:
                                    tmp = spool.tile(
                                        [128, 512], BF16, tag="otmp",
                                        name=f"otmp_{h}_{th}_{cp}_{ci}")
                                    nc.vector.tensor_mul(
                                        tmp[0:HD, :], pcp[0:HD, ci, :],
                                        rbc[0:HD, ci*512:(ci+1)*512])
                                    dst = (o_sb[0:HD, h, tok] if idx == 3
                                           else oacc[0:HD, :])
                                    nc.vector.tensor_add(
                                        dst, oacc[0:HD, :], tmp[0:HD, :])

                        pending_norm[0] = norm
                if pending_norm[0] is not None:
                    pending_norm[0]()
                    pending_norm[0] = None

            # --- proj-C: remaining token tiles (tt 0-3 emitted early) ----------
            emit_projC(range(4, 8))

    nc.compile()
    return nc


def _get_program():
    if "nc" not in _PROGRAM_CACHE:
        _PROGRAM_CACHE["nc"] = _build_program()
    return _PROGRAM_CACHE["nc"]


# ----------------------------------------------------------------------------
# NTFF profiling hook (axon containers without antenv.axon_hooks)
# ----------------------------------------------------------------------------

def _install_ntff_hook():
    """Provide antenv.axon_hooks backed by libaxon_pjrt.so so that
    run_bass_kernel_spmd(trace=True) can capture NTFF profiles under axon.
    Returns True if tracing is possible."""
    try:
        from antenv.axon_hooks import get_axon_ntff_profile_hook  # noqa: F401
        return True
    except ImportError:
        pass
    so_path = "/opt/axon/libaxon_pjrt.so"
    if not os.path.exists(so_path):
        return False
    lib = ctypes.CDLL(so_path)
    if not hasattr(lib, "axon_start_nrt_profile"):
        return False
    lib.axon_start_nrt_profile.argtypes = [
        ctypes.POINTER(ctypes.c_int64), ctypes.c_size_t]
    lib.axon_start_nrt_profile.restype = ctypes.c_int64
    lib.axon_stop_nrt_profile.argtypes = [ctypes.c_char_p]
    lib.axon_stop_nrt_profile.restype = ctypes.c_int64

    @contextlib.contextmanager
    def _hook(output_dir, device_ids):
        import jax
        jax.devices()
        if device_ids:
            ids = (ctypes.c_int64 * len(device_ids))(*device_ids)
            rc = lib.axon_start_nrt_profile(ids, len(device_ids))
        else:
            rc = lib.axon_start_nrt_profile(None, 0)
        if rc != 0:
            raise RuntimeError(f"axon_start_nrt_profile rc={rc}")
        try:
            yield
        finally:
            n = lib.axon_stop_nrt_profile(str(output_dir).encode())
            print(f"profile: {n} file(s) written to {output_dir}",
                  file=sys.stderr)

    mod = types.ModuleType("antenv.axon_hooks")
    _state = {"hook": _hook}
    mod.set_axon_ntff_profile_hook = lambda h: _state.__setitem__("hook", h)
    mod.get_axon_ntff_profile_hook = lambda: _state["hook"]
    sys.modules["antenv.axon_hooks"] = mod
    import antenv
    antenv.axon_hooks = mod
    return True


# ----------------------------------------------------------------------------
# Entry point
# ----------------------------------------------------------------------------

def kernel(trace=False, **inputs):
    nc = _get_program()
    in_maps, bp = _host_prepare(inputs)
    if trace:
        trace = _install_ntff_hook()
    res = bass_utils.run_bass_kernel_spmd(
        nc, in_maps, core_ids=list(range(NCORES)), trace=trace)
    y = np.empty((B, N, DIM), np.float32)
    for b in range(B):
        y[b] = (res.results[2*b]["y"].astype(np.float32)
                + res.results[2*b+1]["y"].astype(np.float32) + bp)
    if trace:
        kernel.last_results = res
    return y

